# revision 4
# baseline (speedup 1.0000x reference)
"""Multi-head attention (B=8, S=1024, D=1024, H=16) on 8 Trainium2 NeuronCores.

Sharding: pure data-parallel over the batch dimension -- core b computes
batch b end-to-end (all 16 heads), so no cross-core collectives are needed.

Per-core plan (all matmuls bf16 with f32 PSUM accumulation):
  1. Load x_q/x_k/x_v and the four weight matrices, transpose 128x128
     blocks on the PE (f32 transpose, cast to bf16 on the PSUM->SBUF copy).
  2. Projections: QT/KT kept transposed [d, s] (bias fused per-partition),
     V kept natural [s, d] with a ones column appended per head (gives the
     softmax row-sum for free during the context matmul).
  3. Per head: scoresT -> exp -> ET (unnormalized, bf16);
     context[sq,dh] + l[sq] = ET.T @ [V_h | 1]; ctx scaled by 1/l;
     scores (natural) -> attn = exp(s/8 + ln(1/l)) fused on ScalarE -> DRAM.
  4. Output projection from DMA-transposed context, rank-1 bias, -> DRAM.
"""

import sys

if "/opt/trn_rl_repo" not in sys.path:
    sys.path.insert(0, "/opt/trn_rl_repo")

import numpy as np

import concourse.bass as bass
import concourse.mybir as mybir
import concourse.tile as tile
from concourse.bass_utils import run_bass_kernel_spmd
from concourse.masks import make_identity
from concourse.tile import ScopedClock

P = 128
S = 1024
D = 1024
H = 16
DH = 64
NB = 8  # batches == cores
NK = D // P  # 8 contraction chunks
NS = S // P  # 8 sequence chunks
SCALE = 0.125  # 1/sqrt(DH)

F32 = mybir.dt.float32
BF16 = mybir.dt.bfloat16


def _install_drain_patch():
    """This walrus build rejects instructions carrying more than one sync
    wait; spread the final TileContext drain's waits across nops."""

    def _drain_and_barrier_chunked(self, tick_clock, wait_clock):
        drain_inst = self.nc.sync.drain()
        wait_clock.add_sem_waits(
            drain_inst.ins, ScopedClock({None: tick_clock.global_clock})
        )
        si = drain_inst.ins.sync_info
        waits = list(si.on_wait)
        if len(waits) > 1:
            drain_inst.ins.sync_info = mybir.SyncInfo(
                on_wait=waits[:1], on_update=list(si.on_update)
            )
            for i in range(1, len(waits)):
                nop = self.nc.sync.nop()
                nop.ins.sync_info = mybir.SyncInfo(
                    on_wait=waits[i : i + 1], on_update=[]
                )
        self.nc.all_engine_barrier()
        popped = self.nc._tile_sem_poison_stack.pop()
        assert popped is self._sem_poison
        self.nc.clear_and_free_semaphores(list(self.sems.allocated().values()))
        self.nc.all_engine_barrier()

    tile.TileContext._drain_and_barrier = _drain_and_barrier_chunked


_install_drain_patch()


def _split_multi_waits(nc: bass.Bass, max_waits: int = 1):
    """This walrus build rejects instructions with more than one sync wait.
    Hoist extra waits onto fresh same-engine nops placed immediately before
    the instruction (same program position => identical semantics)."""
    counter = [0]
    for fn in nc.m.functions:
        for blk in fn.blocks:
            insts = blk.instructions
            new_list = []
            changed = False
            for inst in insts:
                si = getattr(inst, "sync_info", None)
                if si is not None and len(si.on_wait) > max_waits:
                    waits = list(si.on_wait)
                    extra, keep = waits[:-max_waits], waits[-max_waits:]
                    for k in range(0, len(extra), max_waits):
                        nop = mybir.InstNoOp()
                        nop.name = f"waitsplit-{counter[0]}"
                        counter[0] += 1
                        nop.engine = inst.engine
                        nop.sync_info = mybir.SyncInfo(
                            on_wait=extra[k : k + max_waits], on_update=[]
                        )
                        new_list.append(nop)
                    inst.sync_info = mybir.SyncInfo(
                        on_wait=keep, on_update=list(si.on_update)
                    )
                    changed = True
                new_list.append(inst)
            if changed:
                insts[:] = new_list


def build_nc() -> bass.Bass:
    nc = bass.Bass()

    xq = nc.dram_tensor("xq", [S, D], F32, kind="ExternalInput")
    xk = nc.dram_tensor("xk", [S, D], F32, kind="ExternalInput")
    xv = nc.dram_tensor("xv", [S, D], F32, kind="ExternalInput")
    Wq = nc.dram_tensor("Wq", [D, D], F32, kind="ExternalInput")
    Wk = nc.dram_tensor("Wk", [D, D], F32, kind="ExternalInput")
    Wv = nc.dram_tensor("Wv", [D, D], F32, kind="ExternalInput")
    Wo = nc.dram_tensor("Wo", [D, D], F32, kind="ExternalInput")
    bq = nc.dram_tensor("bq", [D], F32, kind="ExternalInput")
    bk = nc.dram_tensor("bk", [D], F32, kind="ExternalInput")
    bv = nc.dram_tensor("bv", [D], F32, kind="ExternalInput")
    bo = nc.dram_tensor("bo", [D], F32, kind="ExternalInput")
    out_d = nc.dram_tensor("out", [S, D], F32, kind="ExternalOutput")
    attn_d = nc.dram_tensor("attn", [H, S, S], F32, kind="ExternalOutput")

    with tile.TileContext(nc) as tc:
        with (
            tc.tile_pool(name="persist", bufs=1) as persist,
            tc.tile_pool(name="pbig", bufs=3, space="PSUM") as pbig,
        ):
            ident = persist.tile([P, P], F32, tag="ident")
            make_identity(nc, ident[:])
            ones_row = persist.tile([1, P], BF16, tag="ones_row")
            nc.vector.memset(ones_row[:], 1.0)
            bor_bf = persist.tile([1, D], BF16, tag="bor_bf")

            WoT = persist.tile([P, NK, D], BF16, tag="WoT")
            QT = persist.tile([P, NK, S], BF16, tag="QT")
            KT = persist.tile([P, NK, S], BF16, tag="KT")
            V = persist.tile([P, NS, H, DH + 1], BF16, tag="V")
            CTX = persist.tile([P, NS, D], BF16, tag="CTX")
            CTXT = persist.tile([P, NK, S], BF16, tag="CTXT")

            # ---- setup + projections ----
            with (
                tc.tile_pool(name="wt", bufs=1) as wt_pool,
                tc.tile_pool(name="stage", bufs=3) as stage_pool,
                tc.tile_pool(name="ptr", bufs=2, space="PSUM") as ptr,
            ):

                def load_T(dram, dest):
                    # dram [R, C] f32 natural; dest[p, cc, r] = dram[r, cc*P+p]
                    for cr in range(NK):
                        st = stage_pool.tile([P, D], F32, tag="stage")
                        nc.sync.dma_start(st[:], dram[cr * P : (cr + 1) * P, :])
                        for ck in range(NK):
                            pt = ptr.tile([P, P], F32, tag="ptr")
                            nc.tensor.transpose(
                                pt[:], st[:, ck * P : (ck + 1) * P], ident[:]
                            )
                            nc.vector.tensor_copy(
                                dest[:, ck, cr * P : (cr + 1) * P], pt[:]
                            )

                WqT = wt_pool.tile([P, NK, D], BF16, tag="WqT")
                WkT = wt_pool.tile([P, NK, D], BF16, tag="WkT")
                WvT = wt_pool.tile([P, NK, D], BF16, tag="WvT")
                bqc = wt_pool.tile([P, NK], F32, tag="bqc")
                nc.sync.dma_start(bqc[:], bq.rearrange("(c p) -> p c", p=P))
                bkc = wt_pool.tile([P, NK], F32, tag="bkc")
                nc.sync.dma_start(bkc[:], bk.rearrange("(c p) -> p c", p=P))
                bvr = wt_pool.tile([1, D], F32, tag="bvr")
                nc.sync.dma_start(bvr[:], bv[None, :])
                bvr_bf = wt_pool.tile([1, D], BF16, tag="bvr_bf")
                nc.vector.tensor_copy(bvr_bf[:], bvr[:])
                bor = wt_pool.tile([1, D], F32, tag="bor")
                nc.sync.dma_start(bor[:], bo[None, :])
                nc.vector.tensor_copy(bor_bf[:], bor[:])

                load_T(Wq, WqT)
                load_T(Wk, WkT)
                load_T(Wv, WvT)
                load_T(Wo, WoT)

                # QT[d, s] = (xq @ Wq.T).T + bq  (bias per partition)
                for proj_w, proj_x_dram, proj_b, dest in (
                    (WqT, xq, bqc, QT),
                    (WkT, xk, bkc, KT),
                ):
                    with tc.tile_pool(name="xt", bufs=1) as xt_pool:
                        xT = xt_pool.tile([P, NK, S], BF16, tag="xT")
                        load_T(proj_x_dram, xT)
                        for cd in range(NK):
                            pq = pbig.tile([P, S], F32, tag="pbig")
                            for j in range(2):
                                for ck in range(NK):
                                    nc.tensor.matmul(
                                        pq[:, j * 512 : (j + 1) * 512],
                                        lhsT=proj_w[:, ck, cd * P : (cd + 1) * P],
                                        rhs=xT[:, ck, j * 512 : (j + 1) * 512],
                                        start=(ck == 0),
                                        stop=(ck == NK - 1),
                                    )
                            nc.vector.tensor_scalar_add(
                                dest[:, cd, :], pq[:], proj_b[:, cd : cd + 1]
                            )

                # V natural [s, d] with bias via rank-1 update
                with tc.tile_pool(name="xt", bufs=1) as xt_pool:
                    xvT = xt_pool.tile([P, NK, S], BF16, tag="xT")
                    load_T(xv, xvT)
                    for cs in range(NS):
                        pv = pbig.tile([P, D], F32, tag="pbig")
                        for j in range(2):
                            for ck in range(NK):
                                nc.tensor.matmul(
                                    pv[:, j * 512 : (j + 1) * 512],
                                    lhsT=xvT[:, ck, cs * P : (cs + 1) * P],
                                    rhs=WvT[:, ck, j * 512 : (j + 1) * 512],
                                    start=(ck == 0),
                                    stop=False,
                                )
                            nc.tensor.matmul(
                                pv[:, j * 512 : (j + 1) * 512],
                                lhsT=ones_row[0:1, :],
                                rhs=bvr_bf[0:1, j * 512 : (j + 1) * 512],
                                start=False,
                                stop=True,
                            )
                        nc.vector.tensor_copy(
                            V[:, cs, :, 0:DH],
                            pv.rearrange("p (h d) -> p h d", h=H),
                        )
            nc.vector.memset(V[:, :, :, DH : DH + 1], 1.0)

            # ---- per-head attention ----
            with (
                tc.tile_pool(name="pet", bufs=2) as pet,
                tc.tile_pool(name="psmall", bufs=20) as psmall,
                tc.tile_pool(name="pattn", bufs=3) as pattn,
                tc.tile_pool(name="pcl", bufs=2, space="PSUM") as pcl,
            ):
                for h in range(H):
                    hp = (h % 2) * DH
                    hc = h // 2
                    ET = pet.tile([P, NS, S], BF16, tag="ET")
                    # A: transposed scores -> unnormalized exp (bf16)
                    for ck in range(NS):
                        ps = pbig.tile([P, S], F32, tag="pbig")
                        for j in range(2):
                            nc.tensor.matmul(
                                ps[:, j * 512 : (j + 1) * 512],
                                lhsT=KT[hp : hp + DH, hc, ck * P : (ck + 1) * P],
                                rhs=QT[hp : hp + DH, hc, j * 512 : (j + 1) * 512],
                                start=True,
                                stop=True,
                            )
                        nc.scalar.activation(
                            ET[:, ck, :],
                            ps[:],
                            mybir.ActivationFunctionType.Exp,
                            scale=SCALE,
                        )
                    # B: context + row-sum via the ones column of V
                    lnrls = []
                    for cq in range(NS):
                        pc = pcl.tile([P, DH + 2], F32, tag="pcl")
                        for ck in range(NS):
                            nc.tensor.matmul(
                                pc[:, 0 : DH + 1],
                                lhsT=ET[:, ck, cq * P : (cq + 1) * P],
                                rhs=V[:, ck, h, :],
                                start=(ck == 0),
                                stop=(ck == NS - 1),
                            )
                        rl = psmall.tile([P, 1], F32, tag="rl")
                        nc.vector.reciprocal(rl[:], pc[:, DH : DH + 1])
                        lnrl = psmall.tile([P, 1], F32, tag="lnrl")
                        nc.scalar.activation(
                            lnrl[:], rl[:], mybir.ActivationFunctionType.Ln
                        )
                        lnrls.append(lnrl)
                        nc.vector.tensor_scalar_mul(
                            CTX[:, cq, h * DH : (h + 1) * DH], pc[:, 0:DH], rl[:]
                        )
                    # C: natural scores -> attn = exp(s/8 + ln(1/l)) -> DRAM
                    for cq in range(NS):
                        ps2 = pbig.tile([P, S], F32, tag="pbig")
                        for j in range(2):
                            nc.tensor.matmul(
                                ps2[:, j * 512 : (j + 1) * 512],
                                lhsT=QT[hp : hp + DH, hc, cq * P : (cq + 1) * P],
                                rhs=KT[hp : hp + DH, hc, j * 512 : (j + 1) * 512],
                                start=True,
                                stop=True,
                            )
                        at = pattn.tile([P, S], F32, tag="attn")
                        nc.scalar.activation(
                            at[:],
                            ps2[:],
                            mybir.ActivationFunctionType.Exp,
                            scale=SCALE,
                            bias=lnrls[cq][:],
                        )
                        nc.sync.dma_start(
                            attn_d[h, cq * P : (cq + 1) * P, :], at[:]
                        )

                # ---- output projection ----
                for cq in range(NS):
                    for cd in range(NK):
                        nc.sync.dma_start_transpose(
                            CTXT[:, cd, cq * P : (cq + 1) * P],
                            CTX[:, cq, cd * P : (cd + 1) * P],
                        )
                for cq in range(NS):
                    po = pbig.tile([P, D], F32, tag="pbig")
                    for j in range(2):
                        for cd in range(NK):
                            nc.tensor.matmul(
                                po[:, j * 512 : (j + 1) * 512],
                                lhsT=CTXT[:, cd, cq * P : (cq + 1) * P],
                                rhs=WoT[:, cd, j * 512 : (j + 1) * 512],
                                start=(cd == 0),
                                stop=False,
                            )
                        nc.tensor.matmul(
                            po[:, j * 512 : (j + 1) * 512],
                            lhsT=ones_row[0:1, :],
                            rhs=bor_bf[0:1, j * 512 : (j + 1) * 512],
                            start=False,
                            stop=True,
                        )
                    osb = pattn.tile([P, D], F32, tag="osb")
                    nc.vector.tensor_copy(osb[:], po[:])
                    nc.sync.dma_start(out_d[cq * P : (cq + 1) * P, :], osb[:])

    _split_multi_waits(nc)
    return nc


_NC_CACHE: dict = {}


def _get_nc() -> bass.Bass:
    if "nc" not in _NC_CACHE:
        _NC_CACHE["nc"] = build_nc()
    return _NC_CACHE["nc"]


def kernel(query, key, value, Wq, bq, Wk, bk, Wv, bv, Wo, bo):
    query = np.ascontiguousarray(np.asarray(query, dtype=np.float32))
    key = np.ascontiguousarray(np.asarray(key, dtype=np.float32))
    value = np.ascontiguousarray(np.asarray(value, dtype=np.float32))
    shared = {
        "Wq": np.ascontiguousarray(np.asarray(Wq, dtype=np.float32)),
        "Wk": np.ascontiguousarray(np.asarray(Wk, dtype=np.float32)),
        "Wv": np.ascontiguousarray(np.asarray(Wv, dtype=np.float32)),
        "Wo": np.ascontiguousarray(np.asarray(Wo, dtype=np.float32)),
        "bq": np.ascontiguousarray(np.asarray(bq, dtype=np.float32)),
        "bk": np.ascontiguousarray(np.asarray(bk, dtype=np.float32)),
        "bv": np.ascontiguousarray(np.asarray(bv, dtype=np.float32)),
        "bo": np.ascontiguousarray(np.asarray(bo, dtype=np.float32)),
    }
    in_maps = [
        {"xq": query[b], "xk": key[b], "xv": value[b], **shared} for b in range(NB)
    ]
    nc = _get_nc()
    res = run_bass_kernel_spmd(nc, in_maps, list(range(NB)))
    output = np.stack([res.results[b]["out"] for b in range(NB)])
    attention = np.stack([res.results[b]["attn"] for b in range(NB)])
    return output, attention


# revision 18
# speedup vs baseline: 1.7442x; 1.7442x over previous
"""Multi-head attention (B=8, S=1024, D=1024, H=16) on 8 Trainium2 NeuronCores.

Sharding: pure data-parallel over the batch dimension -- core b computes
batch b end-to-end (all 16 heads), so no cross-core collectives are needed.

Per-core plan (all matmuls bf16 with f32 PSUM accumulation):
  1. V path first: load Wv/xv, cast bf16 + PE-transpose; V kept natural
     [s, d] with a ones column per head (softmax row-sum falls out of the
     context matmul for free).
  2. Q/K path: loads + transposes up front; the projection chunks
     themselves are emitted just-in-time inside the head pipeline (head h
     only needs d-chunk h//2), so ScalarE starts exponentials early.
  3. Per head (software-pipelined, one-head lookahead):
     A: scoresT -> exp -> ET (unnormalized, bf16)
     B: context[sq,dh] + l[sq] = ET.T @ [V_h | 1]; ctx scaled by 1/l
     C: natural scores -> attn = exp(s/8 + ln(1/l)) fused on ScalarE -> DRAM
  4. Output projection from DMA-transposed context, rank-1 bias, -> DRAM.
"""

import contextlib
import sys

if "/opt/trn_rl_repo" not in sys.path:
    sys.path.insert(0, "/opt/trn_rl_repo")

import numpy as np

import concourse.bass as bass
import concourse.mybir as mybir
import concourse.tile as tile
from concourse.bass_utils import run_bass_kernel_spmd
from concourse.masks import make_identity
from concourse.tile import ScopedClock

P = 128
S = 1024
D = 1024
H = 16
DH = 64
NB = 8  # batches == cores
NK = D // P  # 8 contraction chunks
NS = S // P  # 8 sequence chunks
SCALE = 0.125  # 1/sqrt(DH)

F32 = mybir.dt.float32
BF16 = mybir.dt.bfloat16
EXP = mybir.ActivationFunctionType.Exp
LN = mybir.ActivationFunctionType.Ln


def _install_drain_patch():
    """This walrus build rejects instructions carrying more than one sync
    wait; spread the final TileContext drain's waits across nops."""

    def _drain_and_barrier_chunked(self, tick_clock, wait_clock):
        drain_inst = self.nc.sync.drain()
        wait_clock.add_sem_waits(
            drain_inst.ins, ScopedClock({None: tick_clock.global_clock})
        )
        si = drain_inst.ins.sync_info
        waits = list(si.on_wait) if si is not None else []
        if len(waits) > 1:
            drain_inst.ins.sync_info = mybir.SyncInfo(
                on_wait=waits[:1], on_update=list(si.on_update)
            )
            for i in range(1, len(waits)):
                nop = self.nc.sync.nop()
                nop.ins.sync_info = mybir.SyncInfo(
                    on_wait=waits[i : i + 1], on_update=[]
                )
        self.nc.all_engine_barrier()
        popped = self.nc._tile_sem_poison_stack.pop()
        assert popped is self._sem_poison
        self.nc.clear_and_free_semaphores(list(self.sems.allocated().values()))
        self.nc.all_engine_barrier()

    tile.TileContext._drain_and_barrier = _drain_and_barrier_chunked


_install_drain_patch()


def _split_multi_waits(nc: bass.Bass, max_waits: int = 1):
    """This walrus build rejects instructions with more than one sync wait.
    Hoist extra waits onto fresh same-engine nops placed immediately before
    the instruction (same program position => identical semantics)."""
    counter = [0]
    for fn in nc.m.functions:
        for blk in fn.blocks:
            insts = blk.instructions
            new_list = []
            changed = False
            for inst in insts:
                si = getattr(inst, "sync_info", None)
                if si is not None and len(si.on_wait) > max_waits:
                    waits = list(si.on_wait)
                    extra, keep = waits[:-max_waits], waits[-max_waits:]
                    for k in range(0, len(extra), max_waits):
                        nop = mybir.InstNoOp()
                        nop.name = f"waitsplit-{counter[0]}"
                        counter[0] += 1
                        nop.engine = inst.engine
                        nop.sync_info = mybir.SyncInfo(
                            on_wait=extra[k : k + max_waits], on_update=[]
                        )
                        new_list.append(nop)
                    inst.sync_info = mybir.SyncInfo(
                        on_wait=keep, on_update=list(si.on_update)
                    )
                    changed = True
                new_list.append(inst)
            if changed:
                insts[:] = new_list


def build_nc(repeat: int = 1) -> bass.Bass:
    nc = bass.Bass()

    xq = nc.dram_tensor("xq", [S, D], F32, kind="ExternalInput")
    xk = nc.dram_tensor("xk", [S, D], F32, kind="ExternalInput")
    xv = nc.dram_tensor("xv", [S, D], F32, kind="ExternalInput")
    Wq = nc.dram_tensor("Wq", [D, D], F32, kind="ExternalInput")
    Wk = nc.dram_tensor("Wk", [D, D], F32, kind="ExternalInput")
    Wv = nc.dram_tensor("Wv", [D, D], F32, kind="ExternalInput")
    Wo = nc.dram_tensor("Wo", [D, D], F32, kind="ExternalInput")
    bq = nc.dram_tensor("bq", [D], F32, kind="ExternalInput")
    bk = nc.dram_tensor("bk", [D], F32, kind="ExternalInput")
    bv = nc.dram_tensor("bv", [D], F32, kind="ExternalInput")
    bo = nc.dram_tensor("bo", [D], F32, kind="ExternalInput")
    out_d = nc.dram_tensor("out", [S, D], F32, kind="ExternalOutput")
    attn_d = nc.dram_tensor("attn", [H, S, S], F32, kind="ExternalOutput")

    with tile.TileContext(nc) as tc:
        with (
            tc.For_i(0, repeat, 1) if repeat > 1 else contextlib.nullcontext(),
            tc.tile_pool(name="persist", bufs=1) as persist,
            tc.tile_pool(name="pscore", bufs=3, space="PSUM") as pscore,
        ):
            identf = persist.tile([P, P], F32, tag="identf")
            make_identity(nc, identf[:])
            ident_bf = persist.tile([P, P], BF16, tag="ident_bf")
            make_identity(nc, ident_bf[:])
            ones_row = persist.tile([1, P], BF16, tag="ones_row")
            nc.vector.memset(ones_row[:], 1.0)
            bor_bf = persist.tile([1, D], BF16, tag="bor_bf")

            WoT = persist.tile([P, NK, D], BF16, tag="WoT")
            QT = persist.tile([P, NK, S], BF16, tag="QT")
            KT = persist.tile([P, NK, S], BF16, tag="KT")
            V = persist.tile([P, NS, H, DH + 1], BF16, tag="V")
            CTX = persist.tile([P, NS, D], BF16, tag="CTX")

            def load_T(dram, dest, stage_pool, tpsum):
                """dest[p, cc, r] = dram[r, cc*P+p], bf16: load f32,
                PE-transpose 128x128 f32 blocks, cast to bf16 in the
                4-blocks-per-op PSUM->SBUF copy (DVE)."""
                for cr in range(NK):
                    st = stage_pool.tile([P, D], F32, tag="stage_f32")
                    nc.sync.dma_start(st[:], dram[cr * P : (cr + 1) * P, :])
                    for ck4 in range(2):
                        pt = tpsum.tile([P, 4, P], F32, tag="ptr")
                        for q in range(4):
                            ck = ck4 * 4 + q
                            nc.tensor.transpose(
                                pt[:, q, :], st[:, ck * P : (ck + 1) * P], identf[:]
                            )
                        nc.vector.tensor_copy(
                            dest[:, ck4 * 4 : (ck4 + 1) * 4, cr * P : (cr + 1) * P],
                            pt[:],
                        )

            # ---- V path (scoped pools) ----
            with (
                tc.tile_pool(name="vsetup", bufs=1) as v_pool,
                tc.tile_pool(name="stage1", bufs=4) as stage1,
                tc.tile_pool(name="ptr1", bufs=2, space="PSUM") as ptr1,
            ):
                WvT = v_pool.tile([P, NK, D], BF16, tag="WvT")
                load_T(Wv, WvT, stage1, ptr1)
                bvr = v_pool.tile([1, D], F32, tag="bvr")
                nc.sync.dma_start(bvr[:], bv[None, :])
                bvr_bf = v_pool.tile([1, D], BF16, tag="bvr_bf")
                nc.vector.tensor_copy(bvr_bf[:], bvr[:])
                xvT = v_pool.tile([P, NK, S], BF16, tag="xvT")
                load_T(xv, xvT, stage1, ptr1)

                for cs in range(NS):
                    pv = pscore.tile([P, S], F32, tag="pscore")
                    for j in range(2):
                        pvj = pv[:, j * 512 : (j + 1) * 512]
                        for ck in range(NK):
                            nc.tensor.matmul(
                                pvj,
                                lhsT=xvT[:, ck, cs * P : (cs + 1) * P],
                                rhs=WvT[:, ck, j * 512 : (j + 1) * 512],
                                start=(ck == 0),
                                stop=False,
                            )
                        nc.tensor.matmul(
                            pvj,
                            lhsT=ones_row[0:1, :],
                            rhs=bvr_bf[0:1, j * 512 : (j + 1) * 512],
                            start=False,
                            stop=True,
                        )
                    nc.vector.tensor_copy(
                        V[:, cs, :, 0:DH],
                        pv.rearrange("p (h d) -> p h d", h=H),
                    )
                nc.vector.memset(V[:, :, :, DH : DH + 1], 1.0)

            # ---- Q/K loads + transposes (projection chunks come JIT) ----
            with tc.tile_pool(name="qk", bufs=1) as qk_pool:
                with (
                    tc.tile_pool(name="stage2", bufs=4) as stage2,
                    tc.tile_pool(name="ptr2", bufs=2, space="PSUM") as ptr2,
                ):
                    WqT = qk_pool.tile([P, NK, D], BF16, tag="WqT")
                    load_T(Wq, WqT, stage2, ptr2)
                    xqT = qk_pool.tile([P, NK, S], BF16, tag="xqT")
                    load_T(xq, xqT, stage2, ptr2)
                    WkT = qk_pool.tile([P, NK, D], BF16, tag="WkT")
                    load_T(Wk, WkT, stage2, ptr2)
                    xkT = qk_pool.tile([P, NK, S], BF16, tag="xkT")
                    load_T(xk, xkT, stage2, ptr2)
                    bqc = qk_pool.tile([P, NK], F32, tag="bqc")
                    nc.sync.dma_start(bqc[:], bq.rearrange("(c p) -> p c", p=P))
                    bkc = qk_pool.tile([P, NK], F32, tag="bkc")
                    nc.sync.dma_start(bkc[:], bk.rearrange("(c p) -> p c", p=P))

                def proj_qk(cd):
                    for wT, xT, bc, dest in (
                        (WqT, xqT, bqc, QT),
                        (WkT, xkT, bkc, KT),
                    ):
                        pq = pscore.tile([P, S], F32, tag="pscore")
                        for j in range(2):
                            pqj = pq[:, j * 512 : (j + 1) * 512]
                            for ck in range(NK):
                                nc.tensor.matmul(
                                    pqj,
                                    lhsT=wT[:, ck, cd * P : (cd + 1) * P],
                                    rhs=xT[:, ck, j * 512 : (j + 1) * 512],
                                    start=(ck == 0),
                                    stop=(ck == NK - 1),
                                )
                        nc.vector.tensor_scalar_add(
                            dest[:, cd, :], pq[:], bc[:, cd : cd + 1]
                        )

                # ---- per-head attention, software-pipelined ----
                with (
                    tc.tile_pool(name="pet", bufs=2) as pet,
                    tc.tile_pool(name="psmall", bufs=4) as psmall,
                    tc.tile_pool(name="pattn", bufs=4) as pattn,
                    tc.tile_pool(name="pcl", bufs=2, space="PSUM") as pcl,
                ):
                    ET_tiles = {}

                    def pass_A(h):
                        hp = (h % 2) * DH
                        hc = h // 2
                        ET = pet.tile([P, NS, S], BF16, tag="ET")
                        ET_tiles[h] = ET
                        for ck in range(NS):
                            ps = pscore.tile([P, S], F32, tag="pscore")
                            for j in range(2):
                                nc.tensor.matmul(
                                    ps[:, j * 512 : (j + 1) * 512],
                                    lhsT=KT[
                                        hp : hp + DH, hc, ck * P : (ck + 1) * P
                                    ],
                                    rhs=QT[
                                        hp : hp + DH, hc, j * 512 : (j + 1) * 512
                                    ],
                                    start=True,
                                    stop=True,
                                )
                            nc.scalar.activation(
                                ET[:, ck, :], ps[:], EXP, scale=SCALE
                            )

                    HB = NS // 2  # half-head chunk of cq tiles

                    def pass_B_half(h, rl8, half):
                        ET = ET_tiles[h]
                        for cq in range(half * HB, (half + 1) * HB):
                            pc = pcl.tile([P, DH + 2], F32, tag="pcl")
                            for ck in range(NS):
                                nc.tensor.matmul(
                                    pc[:, 0 : DH + 1],
                                    lhsT=ET[:, ck, cq * P : (cq + 1) * P],
                                    rhs=V[:, ck, h, :],
                                    start=(ck == 0),
                                    stop=(ck == NS - 1),
                                )
                            nc.vector.reciprocal(
                                rl8[:, cq : cq + 1], pc[:, DH : DH + 1]
                            )
                            nc.vector.tensor_scalar_mul(
                                CTX[:, cq, h * DH : (h + 1) * DH],
                                pc[:, 0:DH],
                                rl8[:, cq : cq + 1],
                            )
                        lnrl = psmall.tile([P, HB], F32, tag="lnrl")
                        nc.scalar.activation(
                            lnrl[:], rl8[:, half * HB : (half + 1) * HB], LN
                        )
                        return lnrl

                    def pass_C_half(h, lnrl, half):
                        hp = (h % 2) * DH
                        hc = h // 2
                        for cq in range(half * HB, (half + 1) * HB):
                            ps2 = pscore.tile([P, S], F32, tag="pscore")
                            for j in range(2):
                                nc.tensor.matmul(
                                    ps2[:, j * 512 : (j + 1) * 512],
                                    lhsT=QT[
                                        hp : hp + DH, hc, cq * P : (cq + 1) * P
                                    ],
                                    rhs=KT[
                                        hp : hp + DH, hc, j * 512 : (j + 1) * 512
                                    ],
                                    start=True,
                                    stop=True,
                                )
                            at = pattn.tile([P, S], F32, tag="attn")
                            nc.scalar.activation(
                                at[:],
                                ps2[:],
                                EXP,
                                scale=SCALE,
                                bias=lnrl[:, cq - half * HB : cq - half * HB + 1],
                            )
                            nc.sync.dma_start(
                                attn_d[h, cq * P : (cq + 1) * P, :], at[:]
                            )

                    proj_qk(0)
                    pass_A(0)
                    for h in range(H):
                        if h + 1 < H:
                            if (h + 1) % 2 == 0:
                                proj_qk((h + 1) // 2)
                            pass_A(h + 1)
                        rl8 = psmall.tile([P, NS], F32, tag="rl8")
                        for half in range(2):
                            lnrl = pass_B_half(h, rl8, half)
                            pass_C_half(h, lnrl, half)
                        del ET_tiles[h]

            # ---- Wo load + output projection (late pool) ----
            with tc.tile_pool(name="outp", bufs=1) as outp, tc.tile_pool(
                name="wo_stage", bufs=2
            ) as wo_stage, tc.tile_pool(name="osb_pool", bufs=3) as osb_pool, (
                tc.tile_pool(name="ptr3", bufs=2, space="PSUM")
            ) as ptr3:
                CTXT = outp.tile([P, NK, S], BF16, tag="CTXT")
                load_T(Wo, WoT, wo_stage, ptr3)
                bor = wo_stage.tile([1, D], F32, tag="bor")
                nc.sync.dma_start(bor[:], bo[None, :])
                nc.vector.tensor_copy(bor_bf[:], bor[:])

                for cq in range(NS):
                    for cd4 in range(2):
                        ptc = ptr3.tile([P, 4, P], BF16, tag="ptr")
                        for q in range(4):
                            cd = cd4 * 4 + q
                            nc.tensor.transpose(
                                ptc[:, q, :],
                                CTX[:, cq, cd * P : (cd + 1) * P],
                                ident_bf[:],
                            )
                        nc.vector.tensor_copy(
                            CTXT[
                                :, cd4 * 4 : (cd4 + 1) * 4, cq * P : (cq + 1) * P
                            ],
                            ptc[:],
                        )
                for cq in range(NS):
                    po = pscore.tile([P, S], F32, tag="pscore")
                    for j in range(2):
                        poj = po[:, j * 512 : (j + 1) * 512]
                        for cd in range(NK):
                            nc.tensor.matmul(
                                poj,
                                lhsT=CTXT[:, cd, cq * P : (cq + 1) * P],
                                rhs=WoT[:, cd, j * 512 : (j + 1) * 512],
                                start=(cd == 0),
                                stop=False,
                            )
                        nc.tensor.matmul(
                            poj,
                            lhsT=ones_row[0:1, :],
                            rhs=bor_bf[0:1, j * 512 : (j + 1) * 512],
                            start=False,
                            stop=True,
                        )
                    osb = osb_pool.tile([P, D], F32, tag="osb")
                    nc.vector.tensor_copy(osb[:], po[:])
                    nc.sync.dma_start(out_d[cq * P : (cq + 1) * P, :], osb[:])

    _split_multi_waits(nc)
    return nc


_NC_CACHE: dict = {}


def _get_nc() -> bass.Bass:
    if "nc" not in _NC_CACHE:
        _NC_CACHE["nc"] = build_nc()
    return _NC_CACHE["nc"]


def kernel(query, key, value, Wq, bq, Wk, bk, Wv, bv, Wo, bo):
    query = np.ascontiguousarray(np.asarray(query, dtype=np.float32))
    key = np.ascontiguousarray(np.asarray(key, dtype=np.float32))
    value = np.ascontiguousarray(np.asarray(value, dtype=np.float32))
    shared = {
        "Wq": np.ascontiguousarray(np.asarray(Wq, dtype=np.float32)),
        "Wk": np.ascontiguousarray(np.asarray(Wk, dtype=np.float32)),
        "Wv": np.ascontiguousarray(np.asarray(Wv, dtype=np.float32)),
        "Wo": np.ascontiguousarray(np.asarray(Wo, dtype=np.float32)),
        "bq": np.ascontiguousarray(np.asarray(bq, dtype=np.float32)),
        "bk": np.ascontiguousarray(np.asarray(bk, dtype=np.float32)),
        "bv": np.ascontiguousarray(np.asarray(bv, dtype=np.float32)),
        "bo": np.ascontiguousarray(np.asarray(bo, dtype=np.float32)),
    }
    in_maps = [
        {"xq": query[b], "xk": key[b], "xv": value[b], **shared} for b in range(NB)
    ]
    nc = _get_nc()
    res = run_bass_kernel_spmd(nc, in_maps, list(range(NB)))
    output = np.stack([res.results[b]["out"] for b in range(NB)])
    attention = np.stack([res.results[b]["attn"] for b in range(NB)])
    return output, attention
